# revision 21
# baseline (speedup 1.0000x reference)
"""Trainium2 Bass kernel: gated MSA row attention (AlphaFold-style).

Shapes: q_data/k_data [1,128,256,256], bias [1,8,256,256], k_mask [1,128,256].
Sharding: data-parallel over the 128 sequences -> 16 per core on 8 cores.

Per-core design: channel/key-on-partition layouts so the softmax axis lands on
the PSUM partition dim and the exp-weights come out pre-transposed for the
wavg matmul. Softmax denominators ride along as an extra "ones" column of v
(M=64 wavg matmuls: rows 0-31 wavg, row 32 denom, rows 33-63 zeros).
exp(logits+mask) on ScalarE (no max pass; exp(bias) folded multiplicatively);
reciprocal broadcast via a constant selector matmul; gate & 1/denom applied
elementwise; final projection emits natural [l, o] layout directly.
"""

import os
import sys
import numpy as np
from contextlib import ExitStack

sys.path.insert(0, "/opt/trn_rl_repo")

import concourse.bass as bass
import concourse.bacc as bacc
import concourse.mybir as mybir
from concourse import tile
from concourse.bass_utils import run_bass_kernel_spmd

NCORES = 8
S = 128
SS = S // NCORES          # 16 sequences per core
L = 256                   # residues (q and k length)
C = 256                   # channels
H = 8                     # heads
DK = 32                   # head dim
SCALE = 1.0 / np.sqrt(DK)
MASK_NEG = -30.0          # additive logit offset for masked keys

F32 = mybir.dt.float32
F32R = mybir.dt.float32r
BF16 = mybir.dt.bfloat16
U8 = mybir.dt.uint8
AF = mybir.ActivationFunctionType

OFF_WQ = 0
OFF_WK = OFF_WQ + 512
OFF_WV = OFF_WK + 512
OFF_WG = OFF_WV + 512
OFF_WO = OFF_WG + 1024
OFF_BG = OFF_WO + 1024
OFF_SEL = OFF_BG + 4
OFF_BIAS = OFF_SEL + 128
OFF_BO = OFF_BIAS + 4096
OFF_ID = OFF_BO + 256
NPACK = OFF_ID + 128

# head h -> logits/exp block position; block order [h0,h4 | h1,h5 | h2,h6 | h3,h7]
POS = [2 * (h % 4) + (h // 4) for h in range(8)]
HEAD_AT = [0] * 8
for _h in range(8):
    HEAD_AT[POS[_h]] = _h

_CACHE = {}


def _build_nc():
    nc = bacc.Bacc()

    xqT_e = nc.declare_dram_parameter("xqT", [SS, C, L], F32, isOutput=False)
    xkT_e = nc.declare_dram_parameter("xkT", [SS, C, L], F32, isOutput=False)
    maskT_e = nc.declare_dram_parameter("maskT", [128, 2 * SS], U8, isOutput=False)
    # all f32 constants packed into one [128, NPACK] image (one DMA, one sem)
    pack_e = nc.declare_dram_parameter("pack", [128, NPACK], F32, isOutput=False)
    out_e = nc.declare_dram_parameter("out", [SS * L, 256], F32, isOutput=True)

    with ExitStack() as ctx:
        tc = ctx.enter_context(tile.TileContext(nc))

        # ---------------- pools ----------------
        cpool = ctx.enter_context(tc.tile_pool(name="const", bufs=1))
        xpool = ctx.enter_context(tc.tile_pool(name="x", bufs=3))
        qkpool = ctx.enter_context(tc.tile_pool(name="qk", bufs=3))
        gpool = ctx.enter_context(tc.tile_pool(name="g", bufs=2))
        epool = ctx.enter_context(tc.tile_pool(name="e", bufs=2))
        wpool = ctx.enter_context(tc.tile_pool(name="w", bufs=2))
        opool = ctx.enter_context(tc.tile_pool(name="o", bufs=2))
        ps_proj = ctx.enter_context(tc.tile_pool(name="psproj", bufs=2, space="PSUM"))
        ps_l = ctx.enter_context(tc.tile_pool(name="psl", bufs=1, space="PSUM"))
        ps_w = ctx.enter_context(tc.tile_pool(name="psw", bufs=1, space="PSUM"))
        ps_sm = ctx.enter_context(tc.tile_pool(name="pssm", bufs=2, space="PSUM"))

        cpack = cpool.tile([128, NPACK], F32, name="cpack")
        nc.sync.dma_start(cpack[:], pack_e[:])
        mpack = cpool.tile([128, 2 * SS], U8, name="mpack")
        nc.sync.dma_start(mpack[:], maskT_e[:])

        def _bf(name, off, w):
            t = cpool.tile([128, w], BF16, name=name)
            nc.vector.tensor_copy(t[:], cpack[:, off:off + w])
            return t

        wq_sb = [_bf(f"wqb{kc}", OFF_WQ + 256 * kc, 256) for kc in range(2)]
        wk_sb = [_bf(f"wkb{kc}", OFF_WK + 256 * kc, 256) for kc in range(2)]
        wv_sb = [_bf(f"wvb{kc}", OFF_WV + 256 * kc, 256) for kc in range(2)]
        wg_sb = [_bf(f"wgb{kc}", OFF_WG + 512 * kc, 512) for kc in range(2)]
        ident_sb = _bf("identb", OFF_ID, 128)
        biasb_sb = [_bf(f"biasb{kc}", OFF_BIAS + 2048 * kc, 2048) for kc in range(2)]
        bghalf = cpool.tile([128, 4], F32, name="bghalf")
        nc.vector.tensor_scalar_mul(bghalf[:], cpack[:, OFF_BG:OFF_BG + 4], 0.5)

        # Wo blocks cast to bf16
        wo_sb = []
        for t in range(4):
            b = cpool.tile([128, 256], BF16, name=f"wob{t}")
            nc.vector.tensor_copy(b[:], cpack[:, OFF_WO + 256 * t:OFF_WO + 256 * (t + 1)])
            wo_sb.append(b)

        sel_sb = cpool.tile([128, 128], BF16, name="sel_sb")
        nc.vector.tensor_copy(sel_sb[:], cpack[:, OFF_SEL:OFF_SEL + 128])

        # mask -> additive offsets [128, SS] per k-chunk: mask*30 - 30
        maskadd_sb = []
        for kc in range(2):
            mf = cpool.tile([128, SS], F32, name=f"maskadd{kc}")
            nc.vector.tensor_scalar(
                mf[:], mpack[:, SS * kc:SS * (kc + 1)], -MASK_NEG, MASK_NEG,
                op0=mybir.AluOpType.mult, op1=mybir.AluOpType.add,
            )
            maskadd_sb.append(mf)

        # persistent v tiles [128, 8*64] (per head: 32 v-cols | ones | 31 zeros)
        # double-buffered manually; zeros/ones bands written once.
        NVB = 2
        v_sb = [[cpool.tile([128, 8 * 64], BF16, name=f"vsb{b}_{kc}")
                 for kc in range(2)] for b in range(NVB)]
        for b in range(NVB):
            for kc in range(2):
                t = v_sb[b][kc]
                nc.gpsimd.memset(t[:].rearrange("p (h w) -> p h w", w=64)[:, :, 34:64], 0.0)
                nc.gpsimd.memset(t[:].rearrange("p (h w) -> p h w", w=64)[:, :, 32:34], 1.0)

        for sp in range(SS // 2):
            # -------- load 2 seqs, cast to bf16 (DVE) --------
            xq2 = xpool.tile([128, 4 * L], F32, tag="xq2", name="xq2")
            xk2 = xpool.tile([128, 4 * L], F32, tag="xk2", name="xk2")
            for kc in range(2):
                nc.sync.dma_start(
                    xq2[:, 512 * kc:512 * (kc + 1)].rearrange("p (s l) -> p s l", s=2),
                    xqT_e[2 * sp:2 * sp + 2, 128 * kc:128 * (kc + 1), :].rearrange("s p l -> p s l"))
                nc.sync.dma_start(
                    xk2[:, 512 * kc:512 * (kc + 1)].rearrange("p (s l) -> p s l", s=2),
                    xkT_e[2 * sp:2 * sp + 2, 128 * kc:128 * (kc + 1), :].rearrange("s p l -> p s l"))
            xqb2 = xpool.tile([128, 4 * L], BF16, tag="xqb2", name="xqb2")
            xkb2 = xpool.tile([128, 4 * L], BF16, tag="xkb2", name="xkb2")
            nc.vector.tensor_copy(xqb2[:], xq2[:])
            nc.vector.tensor_copy(xkb2[:], xk2[:])
            # chunk kc of both seqs: cols [512*kc : 512*kc+512]; seq sj at +256*sj

            # -------- q/k projections batched over the pair --------
            qT, kT = [], []
            for m in range(2):
                pq = ps_proj.tile([128, 2 * L], F32, tag="pp", name="pq")
                for kc in range(2):
                    nc.tensor.matmul(
                        pq[:], wq_sb[kc][:, 128 * m:128 * (m + 1)],
                        xqb2[:, 512 * kc:512 * (kc + 1)],
                        start=(kc == 0), stop=(kc == 1),
                    )
                qt = qkpool.tile([128, 2 * L], BF16, tag=f"qT{m}", name=f"qT{m}")
                nc.vector.tensor_scalar_mul(qt[:], pq[:], SCALE)
                qT.append(qt)

                pk = ps_proj.tile([128, 2 * L], F32, tag="pp", name="pk")
                for kc in range(2):
                    nc.tensor.matmul(
                        pk[:], wk_sb[kc][:, 128 * m:128 * (m + 1)],
                        xkb2[:, 512 * kc:512 * (kc + 1)],
                        start=(kc == 0), stop=(kc == 1),
                    )
                kt = qkpool.tile([128, 2 * L], BF16, tag=f"kT{m}", name=f"kT{m}")
                nc.vector.tensor_copy(kt[:], pk[:])
                kT.append(kt)

            for sj in range(2):
                s = 2 * sp + sj
                xq = [xqb2[:, 512 * kc + 256 * sj:512 * kc + 256 * (sj + 1)] for kc in range(2)]
                xk = [xkb2[:, 512 * kc + 256 * sj:512 * kc + 256 * (sj + 1)] for kc in range(2)]

                # v natural [l, hd] into persistent padded tiles
                vcur = v_sb[sj]
                for lc in range(2):
                    pv = ps_proj.tile([128, 256], F32, tag="pp", name="pv")
                    for kc in range(2):
                        nc.tensor.matmul(
                            pv[:], xk[kc][:, 128 * lc:128 * (lc + 1)],
                            wv_sb[kc][:], start=(kc == 0), stop=(kc == 1),
                        )
                    nc.vector.tensor_copy(
                        vcur[lc][:].rearrange("p (h w) -> p h w", w=64)[:, :, 0:32],
                        pv[:].rearrange("p (h w) -> p h w", w=32),
                    )

                # gate pre-activation, permuted 128-row blocks: [128, 4*256]
                gate = gpool.tile([128, 1024], BF16, tag="gate", name="gate")
                gate01 = gpool.tile([128, 1024], BF16, tag="gate01", name="gate01")
                for t in range(4):
                    pgt = ps_sm.tile([128, 256], F32, tag="sm", name="pgt")
                    for kc in range(2):
                        nc.tensor.matmul(
                            pgt[:],
                            wg_sb[kc][:, 128 * t:128 * (t + 1)],
                            xq[kc], start=(kc == 0), stop=(kc == 1),
                        )
                    nc.scalar.activation(
                        gate[:, 256 * t:256 * (t + 1)], pgt[:],
                        AF.Tanh, bias=bghalf[:, t:t + 1], scale=0.5,
                    )

                nc.gpsimd.tensor_scalar(
                    gate01[:], gate[:], 0.5, 0.5,
                    op0=mybir.AluOpType.mult, op1=mybir.AluOpType.add,
                )

                # -------- attention: logits quarters [128, 512] --------
                expT = []
                for kc in range(2):
                    e2 = epool.tile([128, H * L], BF16, tag=f"exp{kc}", name=f"exp{kc}")
                    for qq in range(4):
                        pl = ps_l.tile([128, 512], F32, tag="pl", name="pl")
                        for hh in range(2):
                            nc.tensor.matmul(
                                pl[:, 256 * hh:256 * (hh + 1)], ident_sb[:],
                                biasb_sb[kc][:, 512 * qq + 256 * hh:512 * qq + 256 * (hh + 1)],
                                start=True, stop=False,
                            )
                            # quarter qq holds blocks 2qq, 2qq+1 (heads of ONE
                            # row group -> same-rg MMs serialize; cross-rg MMs
                            # land in the other psum slot/bank)
                            h = HEAD_AT[2 * qq + hh]
                            m, r = h // 4, 32 * (h % 4)
                            nc.tensor.matmul(
                                pl[:, 256 * hh:256 * (hh + 1)],
                                kT[m][r:r + 32, 256 * sj + 128 * kc:256 * sj + 128 * (kc + 1)],
                                qT[m][r:r + 32, 256 * sj:256 * (sj + 1)],
                                start=False, stop=True,
                                tile_position=(r, 0),
                            )
                        nc.scalar.activation(
                            e2[:, 512 * qq:512 * (qq + 1)], pl[:],
                            AF.Exp, bias=maskadd_sb[kc][:, s:s + 1])
                    expT.append(e2)

                # wavg + denominators: psum [128, 4*256]
                pw = ps_w.tile([128, 1024], F32, name="pw")
                for t in range(4):
                    for j in range(2):
                        h = 2 * t + j
                        for kc in range(2):
                            nc.tensor.matmul(
                                pw[64 * j:64 * (j + 1), 256 * t:256 * (t + 1)],
                                vcur[kc][:, 64 * h:64 * (h + 1)],
                                expT[kc][:, 256 * POS[h]:256 * (POS[h] + 1)],
                                start=(kc == 0), stop=(kc == 1),
                                tile_position=(0, 64 * j),
                            )
                wsb = wpool.tile([128, 1024], BF16, tag="wsb", name="wsb")
                nc.vector.tensor_copy(wsb[:], pw[:])

                # broadcast denominators to all rows, then reciprocal
                recipb = wpool.tile([128, 1024], F32, tag="recipb", name="recipb")
                for half in range(2):
                    pdh = ps_sm.tile([128, 512], F32, tag="sm", name="pdh")
                    nc.tensor.matmul(pdh[:], sel_sb[:], wsb[:, 512 * half:512 * (half + 1)],
                                     start=True, stop=True)
                    nc.vector.reciprocal_approx_fast(
                        recipb[:, 512 * half:512 * (half + 1)], pdh[:])

                # gated = wsb * gate01 * recipb
                t1 = wpool.tile([128, 1024], BF16, tag="t1", name="t1")
                nc.gpsimd.tensor_mul(t1[:], wsb[:], gate01[:])
                gated = wpool.tile([128, 1024], BF16, tag="gated", name="gated")
                nc.vector.tensor_mul(gated[:], t1[:], recipb[:])

                # -------- output projection (natural [l, o]) --------
                for lc in range(2):
                    po = ps_sm.tile([128, 256], F32, tag="sm", name="po")
                    for t in range(4):
                        nc.tensor.matmul(
                            po[:], gated[:, 256 * t + 128 * lc:256 * t + 128 * (lc + 1)],
                            wo_sb[t][:], start=(t == 0), stop=(t == 3),
                        )
                    osb = opool.tile([128, 256], F32, tag=f"osb{lc}", name=f"osb{lc}")
                    nc.scalar.copy(osb[:], po[:])
                    nc.sync.dma_start(out_e[L * s + 128 * lc:L * s + 128 * (lc + 1), :], osb[:])

    nc.finalize()
    return nc


def _host_prep(q_data, k_data, bias, k_mask, Wq, Wk, Wv, Wg, bg, Wo, bo):
    """Pure layout transforms (transpose / permute / pad); no arithmetic."""
    q_data = np.ascontiguousarray(np.asarray(q_data, dtype=np.float32))
    k_data = np.ascontiguousarray(np.asarray(k_data, dtype=np.float32))
    bias = np.asarray(bias, dtype=np.float32)
    k_mask = np.asarray(k_mask)

    xqT = np.ascontiguousarray(q_data[0].transpose(0, 2, 1))   # [S, C, L]
    xkT = np.ascontiguousarray(k_data[0].transpose(0, 2, 1))
    biasT_h = bias[0].transpose(2, 0, 1)          # [k, h, q]
    biasT = np.zeros((L, H * L), np.float32)
    for h in range(H):
        biasT[:, 256 * POS[h]:256 * (POS[h] + 1)] = biasT_h[:, h, :]
    maskT_all = np.ascontiguousarray(k_mask[0].astype(np.uint8).T)  # [L, S]

    Wg_ = np.asarray(Wg, dtype=np.float32)
    Wo_ = np.asarray(Wo, dtype=np.float32)
    bg_ = np.asarray(bg, dtype=np.float32)
    wg_p = np.zeros((C, 512), np.float32)
    wo_p = np.zeros((4, 128, 256), np.float32)
    bg_p = np.zeros((4, 128, 1), np.float32)
    for t in range(4):
        for j in range(2):
            h = 2 * t + j
            wg_p[:, 128 * t + 64 * j:128 * t + 64 * j + 32] = Wg_[:, 32 * h:32 * h + 32]
            wo_p[t, 64 * j:64 * j + 32, :] = Wo_[32 * h:32 * h + 32, :]
            bg_p[t, 64 * j:64 * j + 32, 0] = bg_[32 * h:32 * h + 32]
        bg_p[t, 33, 0] = 60.0
        bg_p[t, 97, 0] = 60.0

    wo_p[0, 33, :] = np.asarray(bo, np.float32)
    sel = np.zeros((128, 128), np.float32)
    sel[32, 0:64] = 1.0
    sel[96, 64:128] = 1.0

    pack = np.zeros((128, NPACK), np.float32)
    Wq_ = np.asarray(Wq, np.float32); Wk_ = np.asarray(Wk, np.float32)
    Wv_ = np.asarray(Wv, np.float32)
    for kc in range(2):
        pack[:, OFF_WQ + 256 * kc:OFF_WQ + 256 * (kc + 1)] = Wq_[128 * kc:128 * (kc + 1)]
        pack[:, OFF_WK + 256 * kc:OFF_WK + 256 * (kc + 1)] = Wk_[128 * kc:128 * (kc + 1)]
        pack[:, OFF_WV + 256 * kc:OFF_WV + 256 * (kc + 1)] = Wv_[128 * kc:128 * (kc + 1)]
        pack[:, OFF_WG + 512 * kc:OFF_WG + 512 * (kc + 1)] = wg_p[128 * kc:128 * (kc + 1)]
        pack[:, OFF_BIAS + 2048 * kc:OFF_BIAS + 2048 * (kc + 1)] = biasT[128 * kc:128 * (kc + 1)]
    for t in range(4):
        pack[:, OFF_WO + 256 * t:OFF_WO + 256 * (t + 1)] = wo_p[t]
        pack[:, OFF_BG + t] = bg_p[t, :, 0]
    pack[:, OFF_SEL:OFF_SEL + 128] = sel
    pack[:, OFF_BO:OFF_BO + 256] = np.asarray(bo, np.float32)[None, :]
    pack[:, OFF_ID:OFF_ID + 128] = np.eye(128, dtype=np.float32)

    mask_d = np.zeros((128, 2 * SS), np.uint8)
    common = dict(pack=pack)
    in_maps = []
    for i in range(NCORES):
        m = dict(common)
        m["xqT"] = np.ascontiguousarray(xqT[SS * i:SS * (i + 1)])
        m["xkT"] = np.ascontiguousarray(xkT[SS * i:SS * (i + 1)])
        md = np.zeros((128, 2 * SS), np.uint8)
        mt = maskT_all[:, SS * i:SS * (i + 1)]
        md[:, 0:SS] = mt[0:128]; md[:, SS:2 * SS] = mt[128:256]
        m["maskT"] = md
        in_maps.append(m)
    return in_maps


def kernel(q_data, k_data, bias, k_mask, Wq, Wk, Wv, Wg, bg, Wo, bo):
    in_maps = _host_prep(q_data, k_data, bias, k_mask, Wq, Wk, Wv, Wg, bg, Wo, bo)
    if "nc" not in _CACHE:
        _CACHE["nc"] = _build_nc()
    trace = bool(int(os.environ.get("KERNEL_TRACE", "0")))
    res = run_bass_kernel_spmd(
        _CACHE["nc"], in_maps, core_ids=list(range(NCORES)), trace=trace,
    )
    _CACHE["last_result"] = res
    out = np.concatenate([res.results[i]["out"] for i in range(NCORES)], axis=0)
    return out.reshape(1, S, L, 256)


# revision 22
# speedup vs baseline: 1.2864x; 1.2864x over previous
"""Trainium2 Bass kernel: gated MSA row attention (AlphaFold-style).

Shapes: q_data/k_data [1,128,256,256], bias [1,8,256,256], k_mask [1,128,256].
Sharding: data-parallel over the 128 sequences -> 16 per core on 8 cores.

Per-core design: channel/key-on-partition layouts so the softmax axis lands on
the PSUM partition dim and the exp-weights come out pre-transposed for the
wavg matmul. Softmax denominators ride along as an extra "ones" column of v
(M=64 wavg matmuls: rows 0-31 wavg, row 32 denom, rows 33-63 zeros).
exp(logits+mask) on ScalarE (no max pass; exp(bias) folded multiplicatively);
reciprocal broadcast via a constant selector matmul; gate & 1/denom applied
elementwise; final projection emits natural [l, o] layout directly.
"""

import os
import sys
import numpy as np
from contextlib import ExitStack

sys.path.insert(0, "/opt/trn_rl_repo")

import concourse.bass as bass
import concourse.bacc as bacc
import concourse.mybir as mybir
from concourse import tile
from concourse.bass_utils import run_bass_kernel_spmd

NCORES = 8
S = 128
SS = S // NCORES          # 16 sequences per core
L = 256                   # residues (q and k length)
C = 256                   # channels
H = 8                     # heads
DK = 32                   # head dim
SCALE = 1.0 / np.sqrt(DK)
MASK_NEG = -30.0          # additive logit offset for masked keys

F32 = mybir.dt.float32
F32R = mybir.dt.float32r
BF16 = mybir.dt.bfloat16
U8 = mybir.dt.uint8
AF = mybir.ActivationFunctionType

OFF_WQ = 0
OFF_WK = OFF_WQ + 512
OFF_WV = OFF_WK + 512
OFF_WG = OFF_WV + 512
OFF_WO = OFF_WG + 1024
OFF_BG = OFF_WO + 1024
OFF_SEL = OFF_BG + 4
OFF_BIAS = OFF_SEL + 128
OFF_BO = OFF_BIAS + 4096
OFF_ID = OFF_BO + 256
NPACK = OFF_ID + 128

# head h -> logits/exp block position; block order [h0,h4 | h1,h5 | h2,h6 | h3,h7]
POS = [2 * (h % 4) + (h // 4) for h in range(8)]
HEAD_AT = [0] * 8
for _h in range(8):
    HEAD_AT[POS[_h]] = _h

_CACHE = {}


def _build_nc():
    nc = bacc.Bacc()

    xqT_e = nc.declare_dram_parameter("xqT", [SS, C, L], F32, isOutput=False)
    xkT_e = nc.declare_dram_parameter("xkT", [SS, C, L], F32, isOutput=False)
    maskT_e = nc.declare_dram_parameter("maskT", [128, 2 * SS], U8, isOutput=False)
    # all f32 constants packed into one [128, NPACK] image (one DMA, one sem)
    pack_e = nc.declare_dram_parameter("pack", [128, NPACK], F32, isOutput=False)
    out_e = nc.declare_dram_parameter("out", [SS * L, 256], F32, isOutput=True)

    with ExitStack() as ctx:
        tc = ctx.enter_context(tile.TileContext(nc))

        # ---------------- pools ----------------
        cpool = ctx.enter_context(tc.tile_pool(name="const", bufs=1))
        xpool = ctx.enter_context(tc.tile_pool(name="x", bufs=3))
        qkpool = ctx.enter_context(tc.tile_pool(name="qk", bufs=3))
        gpool = ctx.enter_context(tc.tile_pool(name="g", bufs=2))
        epool = ctx.enter_context(tc.tile_pool(name="e", bufs=2))
        wpool = ctx.enter_context(tc.tile_pool(name="w", bufs=2))
        opool = ctx.enter_context(tc.tile_pool(name="o", bufs=2))
        ps_proj = ctx.enter_context(tc.tile_pool(name="psproj", bufs=2, space="PSUM"))
        ps_l = ctx.enter_context(tc.tile_pool(name="psl", bufs=1, space="PSUM"))
        ps_w = ctx.enter_context(tc.tile_pool(name="psw", bufs=1, space="PSUM"))
        ps_sm = ctx.enter_context(tc.tile_pool(name="pssm", bufs=2, space="PSUM"))

        cpack = cpool.tile([128, NPACK], F32, name="cpack")
        nc.sync.dma_start(cpack[:], pack_e[:])
        mpack = cpool.tile([128, 2 * SS], U8, name="mpack")
        nc.sync.dma_start(mpack[:], maskT_e[:])

        def _bf(name, off, w):
            t = cpool.tile([128, w], BF16, name=name)
            nc.vector.tensor_copy(t[:], cpack[:, off:off + w])
            return t

        wq_sb = [_bf(f"wqb{kc}", OFF_WQ + 256 * kc, 256) for kc in range(2)]
        wk_sb = [_bf(f"wkb{kc}", OFF_WK + 256 * kc, 256) for kc in range(2)]
        wv_sb = [_bf(f"wvb{kc}", OFF_WV + 256 * kc, 256) for kc in range(2)]
        wg_sb = [_bf(f"wgb{kc}", OFF_WG + 512 * kc, 512) for kc in range(2)]
        ident_sb = _bf("identb", OFF_ID, 128)
        biasb_sb = [_bf(f"biasb{kc}", OFF_BIAS + 2048 * kc, 2048) for kc in range(2)]
        bghalf = cpool.tile([128, 4], F32, name="bghalf")
        nc.vector.tensor_scalar_mul(bghalf[:], cpack[:, OFF_BG:OFF_BG + 4], 0.5)

        # Wo blocks cast to bf16
        wo_sb = []
        for t in range(4):
            b = cpool.tile([128, 256], BF16, name=f"wob{t}")
            nc.vector.tensor_copy(b[:], cpack[:, OFF_WO + 256 * t:OFF_WO + 256 * (t + 1)])
            wo_sb.append(b)

        sel_sb = cpool.tile([128, 128], BF16, name="sel_sb")
        nc.vector.tensor_copy(sel_sb[:], cpack[:, OFF_SEL:OFF_SEL + 128])

        # mask -> additive offsets [128, SS] per k-chunk: mask*30 - 30
        maskadd_sb = []
        for kc in range(2):
            mf = cpool.tile([128, SS], F32, name=f"maskadd{kc}")
            nc.vector.tensor_scalar(
                mf[:], mpack[:, SS * kc:SS * (kc + 1)], -MASK_NEG, MASK_NEG,
                op0=mybir.AluOpType.mult, op1=mybir.AluOpType.add,
            )
            maskadd_sb.append(mf)

        # persistent v tiles [128, 8*64] (per head: 32 v-cols | ones | 31 zeros)
        # double-buffered manually; zeros/ones bands written once.
        NVB = 2
        v_sb = [[cpool.tile([128, 8 * 64], BF16, name=f"vsb{b}_{kc}")
                 for kc in range(2)] for b in range(NVB)]
        for b in range(NVB):
            for kc in range(2):
                t = v_sb[b][kc]
                nc.gpsimd.memset(t[:].rearrange("p (h w) -> p h w", w=64)[:, :, 34:64], 0.0)
                nc.gpsimd.memset(t[:].rearrange("p (h w) -> p h w", w=64)[:, :, 32:34], 1.0)

        for s in range(SS):
            # -------- load transposed inputs (one DMA per tensor) --------
            xq2 = xpool.tile([128, 2 * L], F32, tag="xq2", name="xq2")
            xk2 = xpool.tile([128, 2 * L], F32, tag="xk2", name="xk2")
            nc.sync.dma_start(
                xq2[:].rearrange("p (c l) -> p c l", c=2),
                xqT_e[s].rearrange("(c p) l -> p c l", c=2))
            nc.sync.dma_start(
                xk2[:].rearrange("p (c l) -> p c l", c=2),
                xkT_e[s].rearrange("(c p) l -> p c l", c=2))
            xqb2 = xpool.tile([128, 2 * L], BF16, tag="xqb2", name="xqb2")
            xkb2 = xpool.tile([128, 2 * L], BF16, tag="xkb2", name="xkb2")
            nc.vector.tensor_copy(xqb2[:], xq2[:])
            nc.vector.tensor_copy(xkb2[:], xk2[:])
            xq = [xqb2[:, 0:L], xqb2[:, L:2 * L]]
            xk = [xkb2[:, 0:L], xkb2[:, L:2 * L]]

            # -------- projections (bf16) --------
            qT, kT = [], []
            for m in range(2):
                pq = ps_proj.tile([128, L], F32, tag="pp", name="pq")
                for kc in range(2):
                    nc.tensor.matmul(
                        pq[:], wq_sb[kc][:, 128 * m:128 * (m + 1)],
                        xq[kc], start=(kc == 0), stop=(kc == 1),
                    )
                qt = qkpool.tile([128, L], BF16, tag=f"qT{m}", name=f"qT{m}")
                nc.vector.tensor_scalar_mul(qt[:], pq[:], SCALE)
                qT.append(qt)

                pk = ps_proj.tile([128, L], F32, tag="pp", name="pk")
                for kc in range(2):
                    nc.tensor.matmul(
                        pk[:], wk_sb[kc][:, 128 * m:128 * (m + 1)],
                        xk[kc], start=(kc == 0), stop=(kc == 1),
                    )
                kt = qkpool.tile([128, L], BF16, tag=f"kT{m}", name=f"kT{m}")
                nc.vector.tensor_copy(kt[:], pk[:])
                kT.append(kt)

            # v natural [l, hd] into persistent padded tiles
            vcur = v_sb[s % NVB]
            for lc in range(2):
                pv = ps_proj.tile([128, 256], F32, tag="pp", name="pv")
                for kc in range(2):
                    nc.tensor.matmul(
                        pv[:], xk[kc][:, 128 * lc:128 * (lc + 1)],
                        wv_sb[kc][:], start=(kc == 0), stop=(kc == 1),
                    )
                nc.vector.tensor_copy(
                    vcur[lc][:].rearrange("p (h w) -> p h w", w=64)[:, :, 0:32],
                    pv[:].rearrange("p (h w) -> p h w", w=32),
                )

            # gate pre-activation, permuted 128-row blocks: [128, 4*256]
            gate = gpool.tile([128, 1024], BF16, tag="gate", name="gate")
            gate01 = gpool.tile([128, 1024], BF16, tag="gate01", name="gate01")
            for t in range(4):
                pgt = ps_sm.tile([128, 256], F32, tag="sm", name="pgt")
                for kc in range(2):
                    nc.tensor.matmul(
                        pgt[:],
                        wg_sb[kc][:, 128 * t:128 * (t + 1)],
                        xq[kc], start=(kc == 0), stop=(kc == 1),
                    )
                nc.scalar.activation(
                    gate[:, 256 * t:256 * (t + 1)], pgt[:],
                    AF.Tanh, bias=bghalf[:, t:t + 1], scale=0.5,
                )

            nc.gpsimd.tensor_scalar(
                gate01[:], gate[:], 0.5, 0.5,
                op0=mybir.AluOpType.mult, op1=mybir.AluOpType.add,
            )

            # -------- attention --------
            expT = []
            for kc in range(2):
                e2 = epool.tile([128, H * L], BF16, tag=f"exp{kc}", name=f"exp{kc}")
                for half in range(2):
                    pl = ps_l.tile([128, 1024], F32, tag="pl", name="pl")
                    for hh in range(4):
                        nc.tensor.matmul(
                            pl[:, 256 * hh:256 * (hh + 1)], ident_sb[:],
                            biasb_sb[kc][:, 1024 * half + 256 * hh:1024 * half + 256 * (hh + 1)],
                            start=True, stop=False,
                        )
                        # block hh of this half holds head h; chosen so a PSUM
                        # bank only ever holds heads of one row group
                        h = HEAD_AT[4 * half + hh]
                        m, r = h // 4, 32 * (h % 4)
                        nc.tensor.matmul(
                            pl[:, 256 * hh:256 * (hh + 1)],
                            kT[m][r:r + 32, 128 * kc:128 * (kc + 1)],
                            qT[m][r:r + 32, :], start=False, stop=True,
                            tile_position=(r, 0),
                        )
                    nc.scalar.activation(
                        e2[:, 1024 * half:1024 * (half + 1)], pl[:],
                        AF.Exp, bias=maskadd_sb[kc][:, s:s + 1])
                expT.append(e2)

            # wavg + denominators: psum [128, 4*256]
            pw = ps_w.tile([128, 1024], F32, name="pw")
            for t in range(4):
                for j in range(2):
                    h = 2 * t + j
                    for kc in range(2):
                        nc.tensor.matmul(
                            pw[64 * j:64 * (j + 1), 256 * t:256 * (t + 1)],
                            vcur[kc][:, 64 * h:64 * (h + 1)],
                            expT[kc][:, 256 * POS[h]:256 * (POS[h] + 1)],
                            start=(kc == 0), stop=(kc == 1),
                            tile_position=(0, 64 * j),
                        )
            wsb = wpool.tile([128, 1024], BF16, tag="wsb", name="wsb")
            nc.vector.tensor_copy(wsb[:], pw[:])

            # broadcast denominators to all rows, then reciprocal
            recipb = wpool.tile([128, 1024], F32, tag="recipb", name="recipb")
            for half in range(2):
                pdh = ps_sm.tile([128, 512], F32, tag="sm", name="pdh")
                nc.tensor.matmul(pdh[:], sel_sb[:], wsb[:, 512 * half:512 * (half + 1)],
                                 start=True, stop=True)
                nc.vector.reciprocal_approx_fast(
                    recipb[:, 512 * half:512 * (half + 1)], pdh[:])

            # gated = wsb * gate01 * recipb
            t1 = wpool.tile([128, 1024], BF16, tag="t1", name="t1")
            nc.gpsimd.tensor_mul(t1[:], wsb[:], gate01[:])
            gated = wpool.tile([128, 1024], BF16, tag="gated", name="gated")
            nc.vector.tensor_mul(gated[:], t1[:], recipb[:])

            # -------- output projection (natural [l, o]) --------
            for lc in range(2):
                po = ps_sm.tile([128, 256], F32, tag="sm", name="po")
                for t in range(4):
                    nc.tensor.matmul(
                        po[:], gated[:, 256 * t + 128 * lc:256 * t + 128 * (lc + 1)],
                        wo_sb[t][:], start=(t == 0), stop=(t == 3),
                    )
                osb = opool.tile([128, 256], F32, tag=f"osb{lc}", name=f"osb{lc}")
                nc.scalar.copy(osb[:], po[:])
                nc.sync.dma_start(out_e[L * s + 128 * lc:L * s + 128 * (lc + 1), :], osb[:])

    nc.finalize()
    return nc


def _host_prep(q_data, k_data, bias, k_mask, Wq, Wk, Wv, Wg, bg, Wo, bo):
    """Pure layout transforms (transpose / permute / pad); no arithmetic."""
    q_data = np.ascontiguousarray(np.asarray(q_data, dtype=np.float32))
    k_data = np.ascontiguousarray(np.asarray(k_data, dtype=np.float32))
    bias = np.asarray(bias, dtype=np.float32)
    k_mask = np.asarray(k_mask)

    xqT = np.ascontiguousarray(q_data[0].transpose(0, 2, 1))   # [S, C, L]
    xkT = np.ascontiguousarray(k_data[0].transpose(0, 2, 1))
    biasT_h = bias[0].transpose(2, 0, 1)          # [k, h, q]
    biasT = np.zeros((L, H * L), np.float32)
    for h in range(H):
        biasT[:, 256 * POS[h]:256 * (POS[h] + 1)] = biasT_h[:, h, :]
    maskT_all = np.ascontiguousarray(k_mask[0].astype(np.uint8).T)  # [L, S]

    Wg_ = np.asarray(Wg, dtype=np.float32)
    Wo_ = np.asarray(Wo, dtype=np.float32)
    bg_ = np.asarray(bg, dtype=np.float32)
    wg_p = np.zeros((C, 512), np.float32)
    wo_p = np.zeros((4, 128, 256), np.float32)
    bg_p = np.zeros((4, 128, 1), np.float32)
    for t in range(4):
        for j in range(2):
            h = 2 * t + j
            wg_p[:, 128 * t + 64 * j:128 * t + 64 * j + 32] = Wg_[:, 32 * h:32 * h + 32]
            wo_p[t, 64 * j:64 * j + 32, :] = Wo_[32 * h:32 * h + 32, :]
            bg_p[t, 64 * j:64 * j + 32, 0] = bg_[32 * h:32 * h + 32]
        bg_p[t, 33, 0] = 60.0
        bg_p[t, 97, 0] = 60.0

    wo_p[0, 33, :] = np.asarray(bo, np.float32)
    sel = np.zeros((128, 128), np.float32)
    sel[32, 0:64] = 1.0
    sel[96, 64:128] = 1.0

    pack = np.zeros((128, NPACK), np.float32)
    Wq_ = np.asarray(Wq, np.float32); Wk_ = np.asarray(Wk, np.float32)
    Wv_ = np.asarray(Wv, np.float32)
    for kc in range(2):
        pack[:, OFF_WQ + 256 * kc:OFF_WQ + 256 * (kc + 1)] = Wq_[128 * kc:128 * (kc + 1)]
        pack[:, OFF_WK + 256 * kc:OFF_WK + 256 * (kc + 1)] = Wk_[128 * kc:128 * (kc + 1)]
        pack[:, OFF_WV + 256 * kc:OFF_WV + 256 * (kc + 1)] = Wv_[128 * kc:128 * (kc + 1)]
        pack[:, OFF_WG + 512 * kc:OFF_WG + 512 * (kc + 1)] = wg_p[128 * kc:128 * (kc + 1)]
        pack[:, OFF_BIAS + 2048 * kc:OFF_BIAS + 2048 * (kc + 1)] = biasT[128 * kc:128 * (kc + 1)]
    for t in range(4):
        pack[:, OFF_WO + 256 * t:OFF_WO + 256 * (t + 1)] = wo_p[t]
        pack[:, OFF_BG + t] = bg_p[t, :, 0]
    pack[:, OFF_SEL:OFF_SEL + 128] = sel
    pack[:, OFF_BO:OFF_BO + 256] = np.asarray(bo, np.float32)[None, :]
    pack[:, OFF_ID:OFF_ID + 128] = np.eye(128, dtype=np.float32)

    mask_d = np.zeros((128, 2 * SS), np.uint8)
    common = dict(pack=pack)
    in_maps = []
    for i in range(NCORES):
        m = dict(common)
        m["xqT"] = np.ascontiguousarray(xqT[SS * i:SS * (i + 1)])
        m["xkT"] = np.ascontiguousarray(xkT[SS * i:SS * (i + 1)])
        md = np.zeros((128, 2 * SS), np.uint8)
        mt = maskT_all[:, SS * i:SS * (i + 1)]
        md[:, 0:SS] = mt[0:128]; md[:, SS:2 * SS] = mt[128:256]
        m["maskT"] = md
        in_maps.append(m)
    return in_maps


def kernel(q_data, k_data, bias, k_mask, Wq, Wk, Wv, Wg, bg, Wo, bo):
    in_maps = _host_prep(q_data, k_data, bias, k_mask, Wq, Wk, Wv, Wg, bg, Wo, bo)
    if "nc" not in _CACHE:
        _CACHE["nc"] = _build_nc()
    trace = bool(int(os.environ.get("KERNEL_TRACE", "0")))
    res = run_bass_kernel_spmd(
        _CACHE["nc"], in_maps, core_ids=list(range(NCORES)), trace=trace,
    )
    _CACHE["last_result"] = res
    out = np.concatenate([res.results[i]["out"] for i in range(NCORES)], axis=0)
    return out.reshape(1, S, L, 256)
